# revision 3
# baseline (speedup 1.0000x reference)
"""Trainium2 Bass kernel for CovariateAttention (B=2, S=2048, E=1024, 16 heads).

Sharding: 8 cores = 2 (batch) x 4 (head groups of 4 heads).
Per core: q/k/v projections for its 4 heads (tensor-parallel column shard),
RoPE, causal flash-style attention with transposed scores, output projection
row-shard producing a partial [S, E] result; host sums the 4 partials per batch.

v3 layout/speed strategy:
  - q/k projections run in fp8e4 with MatmulPerfMode.DoubleRow: operands are
    packed [128, g, i, *] with the contraction pair i in the free dim, so each
    matmul contracts 256 e-dims at bf16-single-matmul cost (2x throughput).
    Precision sim: 9.5e-3 max-rel (gate 2e-2). v/Wo/PV/scores stay bf16.
  - RoPE uses a 16-interleaved head layout [x1a|x2a|x1b|x2b] so the rotation
    partner swap is a single DVE stream_shuffle per tile (quadrant-local),
    replacing the 4 partition-shift DMAs per tile of v2.
  - exp on the diagonal score tiles is split per k-tile to skip fully-masked
    columns (scalar engine is co-critical with PE in the attention loop).
  - output stores batched to [128, 1024] and issued on sync/gpsimd queues;
    v-ones column memset once at startup instead of DMA per tile.
  - weight/x DMAs split across sync/gpsimd/vector/scalar queues so the first
    projection starts ~5us in instead of ~15us.
"""

import os
import sys

sys.path.insert(0, "/opt/trn_rl_repo")

import numpy as np

N_HEADS = 16
ROPE_BASE = 10000.0
B, S, E = 2, 2048, 1024
D_ATTN = 1024
HDIM = 64
HALF = HDIM // 2
GROUP_HEADS = 4          # heads per core
DL = GROUP_HEADS * HDIM  # 256 local dims per core
N_CORES = 8
ATTN_SCALE = 1.0 / np.sqrt(D_ATTN)

SHUF_MASK = list(range(16, 32)) + list(range(16))

_CACHE = {}


def _build_nc():
    import concourse.tile as tile
    from concourse import bacc, mybir

    f32 = mybir.dt.float32
    dt = mybir.dt.bfloat16
    f8 = mybir.dt.float8e4
    DR = mybir.MatmulPerfMode.DoubleRow

    nc = bacc.Bacc("TRN2", target_bir_lowering=False, debug=False, num_devices=N_CORES)

    xP8_d = nc.dram_tensor("xP8", [128, 4, 4, 2, 512], f8, kind="ExternalInput").ap()
    xP_d = nc.dram_tensor("xP", [128, 4, 2, 4, 512], dt, kind="ExternalInput").ap()
    wq8_d = nc.dram_tensor("wq8", [128, 4, 2, DL], f8, kind="ExternalInput").ap()
    wk8_d = nc.dram_tensor("wk8", [128, 4, 2, DL], f8, kind="ExternalInput").ap()
    wvT_d = nc.dram_tensor("wvT", [E, DL], dt, kind="ExternalInput").ap()
    woT_d = nc.dram_tensor("woT", [DL, E], dt, kind="ExternalInput").ap()
    cos_d = nc.dram_tensor("cosP", [128, S], dt, kind="ExternalInput").ap()
    sin_d = nc.dram_tensor("sinP", [128, S], dt, kind="ExternalInput").ap()
    tri_d = nc.dram_tensor("tri", [128, 128], dt, kind="ExternalInput").ap()
    part_d = nc.dram_tensor("part", [S, E], dt, kind="ExternalOutput").ap()

    NSB = 4    # s-blocks of 512 (projection phase)
    NET = 8    # e-tiles of 128 (bf16 contraction for v)
    NQB = 4    # q-blocks of 512 (attention phase)
    NKT = 16   # k-tiles of 128
    NQT = 16   # q-tiles of 128 (output projection)
    Exp = mybir.ActivationFunctionType.Exp

    with tile.TileContext(nc) as tc:
        with (
            tc.tile_pool(name="weights", bufs=1) as wpool,
            tc.tile_pool(name="persist", bufs=1) as persist,
            tc.tile_pool(name="xin", bufs=4) as xin,
            tc.tile_pool(name="rope", bufs=8) as rope,
            tc.tile_pool(name="probs", bufs=8) as probs,
            tc.tile_pool(name="small", bufs=6) as small,
            tc.tile_pool(name="fout", bufs=4) as fopool,
            tc.tile_pool(name="sc_ps", bufs=2, space="PSUM") as sc_ps,
            tc.tile_pool(name="pv_ps", bufs=4, space="PSUM") as pv_ps,
        ):
            # ---- resident tiles ----
            wq8_sb = wpool.tile([128, 4, 2, DL], f8, tag="wq8")
            wk8_sb = wpool.tile([128, 4, 2, DL], f8, tag="wk8")
            wv_sb = wpool.tile([128, NET, DL], dt, tag="wv")
            wo_sb = wpool.tile([128, 2, E], dt, tag="wo")
            cos_sb = wpool.tile([128, S], dt, tag="cos")
            sin_sb = wpool.tile([128, S], dt, tag="sin")
            tri_sb = wpool.tile([128, 128], dt, tag="tri")
            x8_sb = wpool.tile([128, 4, 4, 2, 512], f8, tag="x8")

            qT = [persist.tile([128, S], dt, tag=f"qT{t}", name=f"qT{t}") for t in range(2)]
            kT = [persist.tile([128, S], dt, tag=f"kT{t}", name=f"kT{t}") for t in range(2)]
            outT = [persist.tile([128, S], dt, tag=f"outT{t}", name=f"outT{t}") for t in range(2)]
            vt = [persist.tile([128, GROUP_HEADS, HDIM + 1], dt, tag=f"vt{i}", name=f"vt{i}")
                  for i in range(NKT)]

            # ---- startup DMA sequencing ----
            def load_x(sb):
                x_sb = xin.tile([128, 8, 512], dt, tag="x", name=f"x{sb}")
                nc.scalar.dma_start(out=x_sb[:, 0:4, :], in_=xP_d[:, sb, 0])
                nc.sync.dma_start(out=x_sb[:, 4:8, :], in_=xP_d[:, sb, 1])
                return x_sb

            nc.sync.dma_start(out=wq8_sb[:], in_=wq8_d)
            nc.gpsimd.dma_start(out=x8_sb[:, 0], in_=xP8_d[:, 0])
            x_first = load_x(0)
            nc.sync.dma_start(out=wk8_sb[:], in_=wk8_d)
            nc.gpsimd.dma_start(out=cos_sb[:], in_=cos_d[:])
            nc.gpsimd.dma_start(out=sin_sb[:], in_=sin_d[:])
            nc.sync.dma_start(out=wv_sb[:], in_=wvT_d.rearrange("(t p) d -> p t d", p=128))
            nc.gpsimd.dma_start(out=x8_sb[:, 1], in_=xP8_d[:, 1])
            x_second = load_x(1)
            nc.sync.dma_start(out=tri_sb[:], in_=tri_d[:])
            nc.gpsimd.dma_start(out=x8_sb[:, 2], in_=xP8_d[:, 2])
            x_third = load_x(2)
            nc.sync.dma_start(out=wo_sb[:], in_=woT_d.rearrange("(t p) e -> p t e", p=128))
            nc.gpsimd.dma_start(out=x8_sb[:, 3], in_=xP8_d[:, 3])
            x_fourth = load_x(3)
            for i in range(NKT):
                nc.gpsimd.memset(vt[i][:, :, HDIM:HDIM + 1], 1.0)

            # ---- Phase A: fp8 DoubleRow q/k projections + RoPE; bf16 v ----
            def phase_a(sb, x_sb):
                ssl = slice(sb * 512, (sb + 1) * 512)
                for dtl in range(2):
                    dsl = slice(dtl * 128, (dtl + 1) * 128)
                    for w8, dest in ((wq8_sb, qT), (wk8_sb, kT)):
                        pp = pv_ps.tile([128, 512], f32, tag="ppv", name=f"pp{sb}{dtl}")
                        for g in range(4):
                            nc.tensor.matmul(
                                pp[:], w8[:, g, :, dsl], x8_sb[:, sb, g],
                                start=(g == 0), stop=(g == 3), perf_mode=DR,
                            )
                        raw = rope.tile([128, 512], dt, tag="raw")
                        nc.scalar.copy(raw[:], pp[:])
                        rot = rope.tile([128, 512], dt, tag="rot")
                        nc.vector.stream_shuffle(rot[:], raw[:], SHUF_MASK)
                        t1 = rope.tile([128, 512], dt, tag="t1")
                        nc.vector.tensor_mul(t1[:], raw[:], cos_sb[:, ssl])
                        t2 = rope.tile([128, 512], dt, tag="t2")
                        nc.vector.tensor_mul(t2[:], rot[:], sin_sb[:, ssl])
                        nc.vector.tensor_add(dest[dtl][:, ssl], t1[:], t2[:])
                # v projection (natural layout [s, d_local]); ones col pre-set
                def xe(et):
                    return x_sb[:, et, :]
                for st in range(4):
                    kt = sb * 4 + st
                    vp = pv_ps.tile([128, DL], f32, tag="ppv", name=f"vp{kt}")
                    for et in range(NET):
                        nc.tensor.matmul(
                            vp[:], xe(et)[:, st * 128:(st + 1) * 128],
                            wv_sb[:, et, :],
                            start=(et == 0), stop=(et == NET - 1),
                        )
                    nc.vector.tensor_copy(
                        vt[kt][:, :, 0:HDIM],
                        vp.rearrange("p (h d) -> p h d", h=GROUP_HEADS),
                    )

            # ---- Phase B: head pairs interleaved, PV lags one k-group ----
            def phase_b(qb):
                qsl = slice(qb * 512, (qb + 1) * 512)
                nkt = 4 * (qb + 1)
                for hp in range(2):
                    t = hp
                    pv = [
                        pv_ps.tile([128, 512], f32, tag="ppv", name=f"pv{qb}{hp}{h2}")
                        for h2 in range(2)
                    ]

                    def make_pv_stage(kp, pr, pv=pv, hp=hp):
                        def emit():
                            for h2 in range(2):
                                h = 2 * hp + h2
                                for j in range(2):
                                    kt = 2 * kp + j
                                    o = max(kt * 128 - qb * 512, 0)
                                    if kt >= 4 * qb:
                                        eng = nc.vector if j == 0 else nc.gpsimd
                                        eng.tensor_mul(
                                            pr[h2][:, j * 512 + o:j * 512 + o + 128],
                                            pr[h2][:, j * 512 + o:j * 512 + o + 128],
                                            tri_sb[:],
                                        )
                                    nc.tensor.matmul(
                                        pv[h2][0:65, o:512], vt[kt][:, h, :],
                                        pr[h2][:, j * 512 + o:(j + 1) * 512],
                                        start=(kt == 0), stop=(kt == nkt - 1),
                                    )
                        return emit

                    pv_prev = None
                    for kp in range(nkt // 2):
                        o0 = max(2 * kp * 128 - qb * 512, 0)
                        o1 = max((2 * kp + 1) * 128 - qb * 512, 0)
                        sc = []
                        for h2 in range(2):
                            psl = slice(h2 * 64, h2 * 64 + 64)
                            s_t = sc_ps.tile([128, 1024], f32, tag="sc")
                            for j in range(2):
                                kt = 2 * kp + j
                                o = max(kt * 128 - qb * 512, 0)
                                nc.tensor.matmul(
                                    s_t[:, j * 512 + o:(j + 1) * 512],
                                    kT[t][psl, kt * 128:(kt + 1) * 128],
                                    qT[t][psl, qb * 512 + o:(qb + 1) * 512],
                                    start=True, stop=True,
                                )
                            sc.append(s_t)
                        pr = []
                        for h2 in range(2):
                            p_t = probs.tile([128, 1024], dt, tag="pr")
                            if o1 > o0:
                                # diagonal round: skip fully-masked cols of j=1
                                nc.scalar.activation(
                                    p_t[:, o0:512], sc[h2][:, o0:512], Exp,
                                    scale=ATTN_SCALE,
                                )
                                nc.scalar.activation(
                                    p_t[:, 512 + o1:], sc[h2][:, 512 + o1:], Exp,
                                    scale=ATTN_SCALE,
                                )
                            else:
                                nc.scalar.activation(
                                    p_t[:, o0:], sc[h2][:, o0:], Exp, scale=ATTN_SCALE
                                )
                            pr.append(p_t)
                        if pv_prev is not None:
                            pv_prev()
                        pv_prev = make_pv_stage(kp, pr)
                    pv_prev()

                    for h2 in range(2):
                        pvs = small.tile([65, 512], f32, tag="pvs", bufs=2)
                        nc.vector.tensor_copy(pvs[:], pv[h2][0:65, :])
                        sums = small.tile([1, 512], f32, tag="sums")
                        nc.vector.tensor_copy(sums[:], pvs[64:65, :])
                        inv = small.tile([1, 512], f32, tag="inv")
                        nc.vector.reciprocal_approx_fast(out=inv[:], in_=sums[:])
                        invb = small.tile([64, 512], f32, tag="invb")
                        nc.gpsimd.partition_broadcast(invb[:], inv[:])
                        nc.vector.tensor_mul(
                            outT[t][h2 * 64 + 0:h2 * 64 + 64, qsl], pvs[0:64, :], invb[:]
                        )

            # ---- Phase C: output projection, batched [128, 1024] stores ----
            def phase_c(qt):
                qsl = slice(qt * 128, (qt + 1) * 128)
                fo = fopool.tile([128, E], dt, tag="fo")
                for eb in range(2):
                    esl = slice(eb * 512, (eb + 1) * 512)
                    f = pv_ps.tile([128, 512], f32, tag="ppv", name=f"f{qt}{eb}")
                    for dtl in range(2):
                        nc.tensor.matmul(
                            f[:], outT[dtl][:, qsl], wo_sb[:, dtl, esl],
                            start=(dtl == 0), stop=(dtl == 1),
                        )
                    if eb == 0:
                        nc.scalar.copy(fo[:, esl], f[:])
                    else:
                        nc.vector.tensor_copy(fo[:, esl], f[:])
                eng = nc.sync if qt % 2 == 0 else nc.gpsimd
                eng.dma_start(out=part_d[qsl, :], in_=fo[:])

            # ---- interleaved emission: A(sb) feeds B(qb=sb); C trails B ----
            phase_a(0, x_first)
            x_pre = [None, x_second, x_third, x_fourth]
            for blk in range(NQB):
                if blk + 1 < NSB:
                    phase_a(blk + 1, x_pre[blk + 1])
                phase_b(blk)
                if blk >= 1:
                    for qt in range(4 * (blk - 1), 4 * blk):
                        phase_c(qt)
            for qt in range(4 * (NQB - 1), NQT):
                phase_c(qt)

    nc.compile()
    return nc


def _host_tables():
    inv_freq = 1.0 / (ROPE_BASE ** (np.arange(HALF, dtype=np.float32) / HALF))
    angles = np.arange(S, dtype=np.float32)[:, None] * inv_freq[None, :]  # [S, 32]
    c = np.cos(angles).T.astype(np.float32)  # [32, S]
    s = np.sin(angles).T.astype(np.float32)
    cos64 = np.concatenate([c[0:16], c[0:16], c[16:32], c[16:32]], axis=0)
    sin64 = np.concatenate([-s[0:16], s[0:16], -s[16:32], s[16:32]], axis=0)
    cosP = np.tile(cos64, (2, 1))  # [128, S]
    sinP = np.tile(sin64, (2, 1))
    tri = (np.arange(128)[None, :] >= np.arange(128)[:, None]).astype(np.float32)
    return cosP, sinP, np.ascontiguousarray(tri)


def kernel(x, Wq, Wk, Wv, Wo):
    import ml_dtypes
    from concourse.bass_utils import run_bass_kernel_spmd

    x = np.asarray(x, dtype=np.float32)
    Wq = np.asarray(Wq, dtype=np.float32)
    Wk = np.asarray(Wk, dtype=np.float32)
    Wv = np.asarray(Wv, dtype=np.float32)
    Wo = np.asarray(Wo, dtype=np.float32)

    if "nc" not in _CACHE:
        _CACHE["nc"] = _build_nc()
    nc = _CACHE["nc"]

    np_bf = ml_dtypes.bfloat16
    np_f8 = ml_dtypes.float8_e4m3fn

    def cvt(a):
        return np.ascontiguousarray(a.astype(np_bf))

    # RoPE 16-interleave permutation within each head:
    # [evens 0:32:2 of pairs 0-15, odds, evens of pairs 16-31, odds]
    e = np.arange(0, HDIM, 2)
    o = np.arange(1, HDIM, 2)
    perm = np.concatenate([e[0:16], o[0:16], e[16:32], o[16:32]])
    full_perm = np.concatenate([h * HDIM + perm for h in range(N_HEADS)])
    Wq_p = Wq[full_perm]
    Wk_p = Wk[full_perm]

    cosP, sinP, tri = _host_tables()

    def packx_bf(xt):
        # [E, S] -> [128, 4, 2, 4, 512]: (p, sb, half, et4, s)
        return np.ascontiguousarray(
            xt.reshape(2, 4, 128, 4, 512).transpose(2, 3, 0, 1, 4))

    def packx_f8(xt):
        # [E, S] -> [128, 4, 4, 2, 512]: (p, sb, g, i, s); e = g*256 + i*128 + p
        return np.ascontiguousarray(
            xt.reshape(4, 2, 128, 4, 512).transpose(2, 3, 0, 1, 4).astype(np_f8))

    def packw8(w_local):
        # w_local [DL, E] -> wT [E, DL] -> [128, 4, 2, DL]
        return np.ascontiguousarray(
            w_local.T.reshape(4, 2, 128, DL).transpose(2, 0, 1, 3).astype(np_f8))

    xT = [np.ascontiguousarray(x[b].T) for b in range(B)]
    xbf = [packx_bf(cvt(xT[b])) for b in range(B)]
    xf8 = [packx_f8(xT[b]) for b in range(B)]
    cosP, sinP, tri = cvt(cosP), cvt(sinP), cvt(tri)

    in_maps = []
    for c in range(N_CORES):
        b, g = c // 4, c % 4
        dsl = slice(g * DL, (g + 1) * DL)
        in_maps.append({
            "xP8": xf8[b],
            "xP": xbf[b],
            "wq8": packw8(Wq_p[dsl]),
            "wk8": packw8(Wk_p[dsl]),
            "wvT": cvt(Wv[dsl].T),
            "woT": cvt(Wo[:, dsl].T),
            "cosP": cosP,
            "sinP": sinP,
            "tri": tri,
        })

    trace = bool(int(os.environ.get("ANT_KERNEL_TRACE", "0")))
    res = None
    for attempt in range(3):
        try:
            res = run_bass_kernel_spmd(
                nc, in_maps, core_ids=list(range(N_CORES)), trace=trace
            )
            break
        except Exception:
            if attempt == 2:
                raise
            import time as _time
            _time.sleep(20)
    _CACHE["last_exec_time_ns"] = res.exec_time_ns
    _CACHE["last_res"] = res

    out = np.zeros((B, S, E), dtype=np.float32)
    for c in range(N_CORES):
        out[c // 4] += np.asarray(res.results[c]["part"], dtype=np.float32)
    return out


# revision 7
# speedup vs baseline: 1.3793x; 1.3793x over previous
"""Trainium2 Bass kernel for CovariateAttention (B=2, S=2048, E=1024, 16 heads).

Sharding: 8 cores = 2 (batch) x 4 (head groups of 4 heads).
Per core: q/k/v projections for its 4 heads (tensor-parallel column shard),
RoPE, causal flash-style attention with transposed scores, output projection
row-shard producing a partial [S, E] result; host sums the 4 partials per batch.

v3 layout/speed strategy:
  - q/k projections run in fp8e4 with MatmulPerfMode.DoubleRow: operands are
    packed [128, g, i, *] with the contraction pair i in the free dim, so each
    matmul contracts 256 e-dims at bf16-single-matmul cost (2x throughput).
    Precision sim: 9.5e-3 max-rel (gate 2e-2). v/Wo/PV/scores stay bf16.
  - RoPE uses a 16-interleaved head layout [x1a|x2a|x1b|x2b] so the rotation
    partner swap is a single DVE stream_shuffle per tile (quadrant-local),
    replacing the 4 partition-shift DMAs per tile of v2.
  - exp on the diagonal score tiles is split per k-tile to skip fully-masked
    columns (scalar engine is co-critical with PE in the attention loop).
  - output stores batched to [128, 1024] and issued on sync/gpsimd queues;
    v-ones column memset once at startup instead of DMA per tile.
  - weight/x DMAs split across sync/gpsimd/vector/scalar queues so the first
    projection starts ~5us in instead of ~15us.
"""

import os
import sys

sys.path.insert(0, "/opt/trn_rl_repo")

import numpy as np

N_HEADS = 16
ROPE_BASE = 10000.0
B, S, E = 2, 2048, 1024
D_ATTN = 1024
HDIM = 64
HALF = HDIM // 2
GROUP_HEADS = 4          # heads per core
DL = GROUP_HEADS * HDIM  # 256 local dims per core
N_CORES = 8
ATTN_SCALE = 1.0 / np.sqrt(D_ATTN)

SHUF_MASK = list(range(16, 32)) + list(range(16))

_CACHE = {}


def _build_nc():
    import concourse.tile as tile
    from concourse import bacc, mybir

    f32 = mybir.dt.float32
    dt = mybir.dt.bfloat16
    f8 = mybir.dt.float8e4
    DR = mybir.MatmulPerfMode.DoubleRow

    nc = bacc.Bacc("TRN2", target_bir_lowering=False, debug=False, num_devices=N_CORES)

    xP8_d = nc.dram_tensor("xP8", [128, 4, 4, 2, 512], f8, kind="ExternalInput").ap()
    xP_d = nc.dram_tensor("xP", [128, 4, 2, 4, 512], dt, kind="ExternalInput").ap()
    wq8_d = nc.dram_tensor("wq8", [128, 4, 2, DL], f8, kind="ExternalInput").ap()
    wk8_d = nc.dram_tensor("wk8", [128, 4, 2, DL], f8, kind="ExternalInput").ap()
    wvT_d = nc.dram_tensor("wvT", [E, DL], dt, kind="ExternalInput").ap()
    woT_d = nc.dram_tensor("woT", [DL, E], dt, kind="ExternalInput").ap()
    cos_d = nc.dram_tensor("cosP", [128, S], dt, kind="ExternalInput").ap()
    sin_d = nc.dram_tensor("sinP", [128, S], dt, kind="ExternalInput").ap()
    tri_d = nc.dram_tensor("tri", [128, 128], dt, kind="ExternalInput").ap()
    part_d = nc.dram_tensor("part", [S, E], dt, kind="ExternalOutput").ap()

    NSB = 4    # s-blocks of 512 (projection phase)
    NET = 8    # e-tiles of 128 (bf16 contraction for v)
    NQB = 4    # q-blocks of 512 (attention phase)
    NKT = 16   # k-tiles of 128
    NQT = 16   # q-tiles of 128 (output projection)
    Exp = mybir.ActivationFunctionType.Exp

    with tile.TileContext(nc) as tc:
        with (
            tc.tile_pool(name="weights", bufs=1) as wpool,
            tc.tile_pool(name="persist", bufs=1) as persist,
            tc.tile_pool(name="xin", bufs=4) as xin,
            tc.tile_pool(name="rope", bufs=8) as rope,
            tc.tile_pool(name="probs", bufs=8) as probs,
            tc.tile_pool(name="small", bufs=6) as small,
            tc.tile_pool(name="fout", bufs=4) as fopool,
            tc.tile_pool(name="sc_ps", bufs=2, space="PSUM") as sc_ps,
            tc.tile_pool(name="pv_ps", bufs=4, space="PSUM") as pv_ps,
        ):
            # ---- resident tiles ----
            wq8_sb = wpool.tile([128, 4, 2, DL], f8, tag="wq8")
            wk8_sb = wpool.tile([128, 4, 2, DL], f8, tag="wk8")
            wv_sb = wpool.tile([128, NET, DL], dt, tag="wv")
            wo_sb = wpool.tile([128, 2, E], dt, tag="wo")
            cos_sb = wpool.tile([128, S], dt, tag="cos")
            sin_sb = wpool.tile([128, S], dt, tag="sin")
            tri_sb = wpool.tile([128, 128], dt, tag="tri")
            x8_sb = wpool.tile([128, 4, 4, 2, 512], f8, tag="x8")

            qT = [persist.tile([128, S], dt, tag=f"qT{t}", name=f"qT{t}") for t in range(2)]
            kT = [persist.tile([128, S], dt, tag=f"kT{t}", name=f"kT{t}") for t in range(2)]
            outT = [persist.tile([128, S], dt, tag=f"outT{t}", name=f"outT{t}") for t in range(2)]
            vt = [persist.tile([128, GROUP_HEADS, HDIM + 1], dt, tag=f"vt{i}", name=f"vt{i}")
                  for i in range(NKT)]

            # ---- startup DMA sequencing ----
            def load_x(sb):
                x_sb = xin.tile([128, 8, 512], dt, tag="x", name=f"x{sb}")
                nc.scalar.dma_start(out=x_sb[:, 0:4, :], in_=xP_d[:, sb, 0])
                nc.sync.dma_start(out=x_sb[:, 4:8, :], in_=xP_d[:, sb, 1])
                return x_sb

            nc.sync.dma_start(out=wq8_sb[:], in_=wq8_d)
            nc.gpsimd.dma_start(out=x8_sb[:, 0], in_=xP8_d[:, 0])
            x_first = load_x(0)
            nc.sync.dma_start(out=wk8_sb[:], in_=wk8_d)
            nc.gpsimd.dma_start(out=cos_sb[:], in_=cos_d[:])
            nc.gpsimd.dma_start(out=sin_sb[:], in_=sin_d[:])
            nc.sync.dma_start(out=wv_sb[:], in_=wvT_d.rearrange("(t p) d -> p t d", p=128))
            nc.gpsimd.dma_start(out=x8_sb[:, 1], in_=xP8_d[:, 1])
            x_second = load_x(1)
            nc.sync.dma_start(out=tri_sb[:], in_=tri_d[:])
            nc.gpsimd.dma_start(out=x8_sb[:, 2], in_=xP8_d[:, 2])
            x_third = load_x(2)
            nc.sync.dma_start(out=wo_sb[:], in_=woT_d.rearrange("(t p) e -> p t e", p=128))
            nc.gpsimd.dma_start(out=x8_sb[:, 3], in_=xP8_d[:, 3])
            x_fourth = load_x(3)
            for i in range(NKT):
                nc.gpsimd.memset(vt[i][:, :, HDIM:HDIM + 1], 1.0)

            # ---- Phase A: fp8 DoubleRow q/k projections + RoPE; bf16 v ----
            def phase_a(sb, x_sb):
                ssl = slice(sb * 512, (sb + 1) * 512)
                for dtl in range(2):
                    dsl = slice(dtl * 128, (dtl + 1) * 128)
                    for w8, dest in ((wq8_sb, qT), (wk8_sb, kT)):
                        pp = pv_ps.tile([128, 512], f32, tag="ppv", name=f"pp{sb}{dtl}")
                        for g in range(4):
                            nc.tensor.matmul(
                                pp[:], w8[:, g, :, dsl], x8_sb[:, sb, g],
                                start=(g == 0), stop=(g == 3), perf_mode=DR,
                            )
                        raw = rope.tile([128, 512], dt, tag="raw")
                        nc.scalar.copy(raw[:], pp[:])
                        rot = rope.tile([128, 512], dt, tag="rot")
                        nc.vector.stream_shuffle(rot[:], raw[:], SHUF_MASK)
                        t1 = rope.tile([128, 512], dt, tag="t1")
                        nc.vector.tensor_mul(t1[:], raw[:], cos_sb[:, ssl])
                        t2 = rope.tile([128, 512], dt, tag="t2")
                        nc.vector.tensor_mul(t2[:], rot[:], sin_sb[:, ssl])
                        nc.vector.tensor_add(dest[dtl][:, ssl], t1[:], t2[:])
                # v projection (natural layout [s, d_local]); ones col pre-set
                def xe(et):
                    return x_sb[:, et, :]
                for st in range(4):
                    kt = sb * 4 + st
                    vp = pv_ps.tile([128, DL], f32, tag="ppv", name=f"vp{kt}")
                    for et in range(NET):
                        nc.tensor.matmul(
                            vp[:], xe(et)[:, st * 128:(st + 1) * 128],
                            wv_sb[:, et, :],
                            start=(et == 0), stop=(et == NET - 1),
                        )
                    nc.vector.tensor_copy(
                        vt[kt][:, :, 0:HDIM],
                        vp.rearrange("p (h d) -> p h d", h=GROUP_HEADS),
                    )

            # ---- Phase B: head pairs interleaved, PV lags one k-group ----
            def phase_b(qb):
                qsl = slice(qb * 512, (qb + 1) * 512)
                nkt = 4 * (qb + 1)
                for hp in range(2):
                    t = hp
                    pv = [
                        pv_ps.tile([128, 512], f32, tag="ppv", name=f"pv{qb}{hp}{h2}")
                        for h2 in range(2)
                    ]

                    def make_pv_stage(kp, pr, pv=pv, hp=hp):
                        def emit():
                            for h2 in range(2):
                                h = 2 * hp + h2
                                for j in range(2):
                                    kt = 2 * kp + j
                                    o = max(kt * 128 - qb * 512, 0)
                                    if kt >= 4 * qb:
                                        nc.vector.tensor_mul(
                                            pr[h2][:, j * 512 + o:j * 512 + o + 128],
                                            pr[h2][:, j * 512 + o:j * 512 + o + 128],
                                            tri_sb[:],
                                        )
                                    nc.tensor.matmul(
                                        pv[h2][0:65, o:512], vt[kt][:, h, :],
                                        pr[h2][:, j * 512 + o:(j + 1) * 512],
                                        start=(kt == 0), stop=(kt == nkt - 1),
                                    )
                        return emit

                    pv_prev = None
                    for kp in range(nkt // 2):
                        o0 = max(2 * kp * 128 - qb * 512, 0)
                        sc = []
                        for h2 in range(2):
                            psl = slice(h2 * 64, h2 * 64 + 64)
                            s_t = sc_ps.tile([128, 1024], f32, tag="sc")
                            for j in range(2):
                                kt = 2 * kp + j
                                o = max(kt * 128 - qb * 512, 0)
                                nc.tensor.matmul(
                                    s_t[:, j * 512 + o:(j + 1) * 512],
                                    kT[t][psl, kt * 128:(kt + 1) * 128],
                                    qT[t][psl, qb * 512 + o:(qb + 1) * 512],
                                    start=True, stop=True,
                                )
                            sc.append(s_t)
                        pr = []
                        for h2 in range(2):
                            p_t = probs.tile([128, 1024], dt, tag="pr")
                            nc.scalar.activation(
                                p_t[:, o0:], sc[h2][:, o0:], Exp, scale=ATTN_SCALE
                            )
                            pr.append(p_t)
                        if pv_prev is not None:
                            pv_prev()
                        pv_prev = make_pv_stage(kp, pr)
                    pv_prev()

                    for h2 in range(2):
                        sums = small.tile([1, 512], f32, tag="sums")
                        nc.vector.tensor_copy(sums[:], pv[h2][64:65, :])
                        inv = small.tile([1, 512], f32, tag="inv")
                        nc.vector.reciprocal_approx_fast(out=inv[:], in_=sums[:])
                        invb = small.tile([64, 512], f32, tag="invb")
                        nc.gpsimd.partition_broadcast(invb[:], inv[:])
                        nc.vector.tensor_mul(
                            outT[t][h2 * 64 + 0:h2 * 64 + 64, qsl],
                            pv[h2][0:64, :], invb[:]
                        )

            # ---- Phase C: output projection, batched [128, 1024] stores ----
            def phase_c(qt):
                qsl = slice(qt * 128, (qt + 1) * 128)
                fo = fopool.tile([128, E], dt, tag="fo")
                for eb in range(2):
                    esl = slice(eb * 512, (eb + 1) * 512)
                    f = pv_ps.tile([128, 512], f32, tag="ppv", name=f"f{qt}{eb}")
                    for dtl in range(2):
                        nc.tensor.matmul(
                            f[:], outT[dtl][:, qsl], wo_sb[:, dtl, esl],
                            start=(dtl == 0), stop=(dtl == 1),
                        )
                    if eb == 0:
                        nc.scalar.copy(fo[:, esl], f[:])
                    else:
                        nc.vector.tensor_copy(fo[:, esl], f[:])
                eng = nc.sync if qt % 2 == 0 else nc.gpsimd
                eng.dma_start(out=part_d[qsl, :], in_=fo[:])

            # ---- interleaved emission: A(sb) feeds B(qb=sb); C trails B ----
            phase_a(0, x_first)
            x_pre = [None, x_second, x_third, x_fourth]
            for blk in range(NQB):
                if blk + 1 < NSB:
                    phase_a(blk + 1, x_pre[blk + 1])
                phase_b(blk)
                if blk >= 1:
                    for qt in range(4 * (blk - 1), 4 * blk):
                        phase_c(qt)
            for qt in range(4 * (NQB - 1), NQT):
                phase_c(qt)

    nc.compile()
    return nc


def _host_tables():
    inv_freq = 1.0 / (ROPE_BASE ** (np.arange(HALF, dtype=np.float32) / HALF))
    angles = np.arange(S, dtype=np.float32)[:, None] * inv_freq[None, :]  # [S, 32]
    c = np.cos(angles).T.astype(np.float32)  # [32, S]
    s = np.sin(angles).T.astype(np.float32)
    cos64 = np.concatenate([c[0:16], c[0:16], c[16:32], c[16:32]], axis=0)
    sin64 = np.concatenate([-s[0:16], s[0:16], -s[16:32], s[16:32]], axis=0)
    cosP = np.tile(cos64, (2, 1))  # [128, S]
    sinP = np.tile(sin64, (2, 1))
    tri = (np.arange(128)[None, :] >= np.arange(128)[:, None]).astype(np.float32)
    return cosP, sinP, np.ascontiguousarray(tri)


def kernel(x, Wq, Wk, Wv, Wo):
    import ml_dtypes
    from concourse.bass_utils import run_bass_kernel_spmd

    x = np.asarray(x, dtype=np.float32)
    Wq = np.asarray(Wq, dtype=np.float32)
    Wk = np.asarray(Wk, dtype=np.float32)
    Wv = np.asarray(Wv, dtype=np.float32)
    Wo = np.asarray(Wo, dtype=np.float32)

    if "nc" not in _CACHE:
        _CACHE["nc"] = _build_nc()
    nc = _CACHE["nc"]

    np_bf = ml_dtypes.bfloat16
    np_f8 = ml_dtypes.float8_e4m3fn

    def cvt(a):
        return np.ascontiguousarray(a.astype(np_bf))

    # RoPE 16-interleave permutation within each head:
    # [evens 0:32:2 of pairs 0-15, odds, evens of pairs 16-31, odds]
    e = np.arange(0, HDIM, 2)
    o = np.arange(1, HDIM, 2)
    perm = np.concatenate([e[0:16], o[0:16], e[16:32], o[16:32]])
    full_perm = np.concatenate([h * HDIM + perm for h in range(N_HEADS)])
    Wq_p = Wq[full_perm]
    Wk_p = Wk[full_perm]

    cosP, sinP, tri = _host_tables()

    def packx_bf(xt):
        # [E, S] -> [128, 4, 2, 4, 512]: (p, sb, half, et4, s)
        return np.ascontiguousarray(
            xt.reshape(2, 4, 128, 4, 512).transpose(2, 3, 0, 1, 4))

    def packx_f8(xt):
        # [E, S] -> [128, 4, 4, 2, 512]: (p, sb, g, i, s); e = g*256 + i*128 + p
        return np.ascontiguousarray(
            xt.reshape(4, 2, 128, 4, 512).transpose(2, 3, 0, 1, 4).astype(np_f8))

    def packw8(w_local):
        # w_local [DL, E] -> wT [E, DL] -> [128, 4, 2, DL]
        return np.ascontiguousarray(
            w_local.T.reshape(4, 2, 128, DL).transpose(2, 0, 1, 3).astype(np_f8))

    xT = [np.ascontiguousarray(x[b].T) for b in range(B)]
    xbf = [packx_bf(cvt(xT[b])) for b in range(B)]
    xf8 = [packx_f8(xT[b]) for b in range(B)]
    cosP, sinP, tri = cvt(cosP), cvt(sinP), cvt(tri)

    in_maps = []
    for c in range(N_CORES):
        b, g = c // 4, c % 4
        dsl = slice(g * DL, (g + 1) * DL)
        in_maps.append({
            "xP8": xf8[b],
            "xP": xbf[b],
            "wq8": packw8(Wq_p[dsl]),
            "wk8": packw8(Wk_p[dsl]),
            "wvT": cvt(Wv[dsl].T),
            "woT": cvt(Wo[:, dsl].T),
            "cosP": cosP,
            "sinP": sinP,
            "tri": tri,
        })

    trace = bool(int(os.environ.get("ANT_KERNEL_TRACE", "0")))
    res = None
    for attempt in range(3):
        try:
            res = run_bass_kernel_spmd(
                nc, in_maps, core_ids=list(range(N_CORES)), trace=trace
            )
            break
        except Exception:
            if attempt == 2:
                raise
            import time as _time
            _time.sleep(20)
    _CACHE["last_exec_time_ns"] = res.exec_time_ns
    _CACHE["last_res"] = res

    out = np.zeros((B, S, E), dtype=np.float32)
    for c in range(N_CORES):
        out[c // 4] += np.asarray(res.results[c]["part"], dtype=np.float32)
    return out


# revision 11
# speedup vs baseline: 1.3860x; 1.0049x over previous
"""Trainium2 Bass kernel for CovariateAttention (B=2, S=2048, E=1024, 16 heads).

Sharding: 8 cores = 2 (batch) x 4 (head groups of 4 heads).
Per core: q/k/v projections for its 4 heads (tensor-parallel column shard),
RoPE, causal flash-style attention with transposed scores, output projection
row-shard producing a partial [S, E] result; host sums the 4 partials per batch.

v3 layout/speed strategy:
  - q/k projections run in fp8e4 with MatmulPerfMode.DoubleRow: operands are
    packed [128, g, i, *] with the contraction pair i in the free dim, so each
    matmul contracts 256 e-dims at bf16-single-matmul cost (2x throughput).
    Precision sim: 9.5e-3 max-rel (gate 2e-2). v/Wo/PV/scores stay bf16.
  - RoPE uses a 16-interleaved head layout [x1a|x2a|x1b|x2b] so the rotation
    partner swap is a single DVE stream_shuffle per tile (quadrant-local),
    replacing the 4 partition-shift DMAs per tile of v2.
  - exp on the diagonal score tiles is split per k-tile to skip fully-masked
    columns (scalar engine is co-critical with PE in the attention loop).
  - output stores batched to [128, 1024] and issued on sync/gpsimd queues;
    v-ones column memset once at startup instead of DMA per tile.
  - weight/x DMAs split across sync/gpsimd/vector/scalar queues so the first
    projection starts ~5us in instead of ~15us.
"""

import os
import sys

sys.path.insert(0, "/opt/trn_rl_repo")

import numpy as np

N_HEADS = 16
ROPE_BASE = 10000.0
B, S, E = 2, 2048, 1024
D_ATTN = 1024
HDIM = 64
HALF = HDIM // 2
GROUP_HEADS = 4          # heads per core
DL = GROUP_HEADS * HDIM  # 256 local dims per core
N_CORES = 8
ATTN_SCALE = 1.0 / np.sqrt(D_ATTN)

SHUF_MASK = list(range(16, 32)) + list(range(16))

_CACHE = {}


def _build_nc():
    import concourse.tile as tile
    from concourse import bacc, mybir

    f32 = mybir.dt.float32
    dt = mybir.dt.bfloat16
    f8 = mybir.dt.float8e4
    DR = mybir.MatmulPerfMode.DoubleRow

    nc = bacc.Bacc("TRN2", target_bir_lowering=False, debug=False, num_devices=N_CORES)

    xP8_d = nc.dram_tensor("xP8", [128, 4, 4, 2, 512], f8, kind="ExternalInput").ap()
    xP_d = nc.dram_tensor("xP", [128, 4, 2, 4, 512], dt, kind="ExternalInput").ap()
    wq8_d = nc.dram_tensor("wq8", [128, 4, 2, DL], f8, kind="ExternalInput").ap()
    wk8_d = nc.dram_tensor("wk8", [128, 4, 2, DL], f8, kind="ExternalInput").ap()
    wvT_d = nc.dram_tensor("wvT", [E, DL], dt, kind="ExternalInput").ap()
    woT_d = nc.dram_tensor("woT", [DL, E], dt, kind="ExternalInput").ap()
    cos_d = nc.dram_tensor("cosP", [128, S], dt, kind="ExternalInput").ap()
    sin_d = nc.dram_tensor("sinP", [128, S], dt, kind="ExternalInput").ap()
    tri_d = nc.dram_tensor("tri", [128, 128], dt, kind="ExternalInput").ap()
    part_d = nc.dram_tensor("part", [S, E], dt, kind="ExternalOutput").ap()

    NSB = 4    # s-blocks of 512 (projection phase)
    NET = 8    # e-tiles of 128 (bf16 contraction for v)
    NQB = 4    # q-blocks of 512 (attention phase)
    NKT = 16   # k-tiles of 128
    NQT = 16   # q-tiles of 128 (output projection)
    Exp = mybir.ActivationFunctionType.Exp

    with tile.TileContext(nc) as tc:
        with (
            tc.tile_pool(name="weights", bufs=1) as wpool,
            tc.tile_pool(name="persist", bufs=1) as persist,
            tc.tile_pool(name="xin", bufs=4) as xin,
            tc.tile_pool(name="rope", bufs=8) as rope,
            tc.tile_pool(name="probs", bufs=8) as probs,
            tc.tile_pool(name="small", bufs=6) as small,
            tc.tile_pool(name="fout", bufs=4) as fopool,
            tc.tile_pool(name="sc_ps", bufs=2, space="PSUM") as sc_ps,
            tc.tile_pool(name="pv_ps", bufs=4, space="PSUM") as pv_ps,
        ):
            # ---- resident tiles ----
            # wq8/wk8 split into g01/g23 halves so the first projection
            # chain can start as soon as its half lands (separate DMA rings)
            wq8_sb = [wpool.tile([128, 2, 2, DL], f8, tag=f"wq8{i}", name=f"wq8{i}")
                      for i in range(2)]
            wk8_sb = [wpool.tile([128, 2, 2, DL], f8, tag=f"wk8{i}", name=f"wk8{i}")
                      for i in range(2)]
            wv_sb = wpool.tile([128, NET, DL], dt, tag="wv")
            wo_sb = wpool.tile([128, 2, E], dt, tag="wo")
            cos_sb = wpool.tile([128, S], dt, tag="cos")
            sin_sb = wpool.tile([128, S], dt, tag="sin")
            tri_sb = wpool.tile([128, 128], dt, tag="tri")
            x8_sb = wpool.tile([128, 4, 4, 2, 512], f8, tag="x8")

            qT = [persist.tile([128, S], dt, tag=f"qT{t}", name=f"qT{t}") for t in range(2)]
            kT = [persist.tile([128, S], dt, tag=f"kT{t}", name=f"kT{t}") for t in range(2)]
            outT = [persist.tile([128, S], dt, tag=f"outT{t}", name=f"outT{t}") for t in range(2)]
            vt = [persist.tile([128, GROUP_HEADS, HDIM + 1], dt, tag=f"vt{i}", name=f"vt{i}")
                  for i in range(NKT)]

            # ---- startup DMA sequencing: chunk critical loads across DMA
            # rings (a single DMA rides one ring at ~25GB/s) and spread the
            # ~1us issue cost across the sync/gpsimd/scalar queues ----
            def load_x(sb, chunks=2):
                x_sb = xin.tile([128, 8, 512], dt, tag="x", name=f"x{sb}")
                if chunks == 4:
                    nc.scalar.dma_start(out=x_sb[:, 0:2, :], in_=xP_d[:, sb, 0, 0:2])
                    nc.scalar.dma_start(out=x_sb[:, 2:4, :], in_=xP_d[:, sb, 0, 2:4])
                    nc.scalar.dma_start(out=x_sb[:, 4:6, :], in_=xP_d[:, sb, 1, 0:2])
                    nc.scalar.dma_start(out=x_sb[:, 6:8, :], in_=xP_d[:, sb, 1, 2:4])
                else:
                    nc.scalar.dma_start(out=x_sb[:, 0:4, :], in_=xP_d[:, sb, 0])
                    nc.scalar.dma_start(out=x_sb[:, 4:8, :], in_=xP_d[:, sb, 1])
                return x_sb

            wvT_r = wvT_d.rearrange("(t p) d -> p t d", p=128)
            woT_r = woT_d.rearrange("(t p) e -> p t e", p=128)
            nc.sync.dma_start(out=wq8_sb[0][:], in_=wq8_d[:, 0:2])
            nc.sync.dma_start(out=wq8_sb[1][:], in_=wq8_d[:, 2:4])
            nc.gpsimd.dma_start(out=x8_sb[:, 0, 0:2], in_=xP8_d[:, 0, 0:2])
            nc.gpsimd.dma_start(out=x8_sb[:, 0, 2:4], in_=xP8_d[:, 0, 2:4])
            x_first = load_x(0, chunks=4)
            nc.sync.dma_start(out=wk8_sb[0][:], in_=wk8_d[:, 0:2])
            nc.sync.dma_start(out=wk8_sb[1][:], in_=wk8_d[:, 2:4])
            nc.gpsimd.dma_start(out=cos_sb[:, 0:1024], in_=cos_d[:, 0:1024])
            nc.gpsimd.dma_start(out=cos_sb[:, 1024:2048], in_=cos_d[:, 1024:2048])
            nc.gpsimd.dma_start(out=sin_sb[:, 0:1024], in_=sin_d[:, 0:1024])
            nc.gpsimd.dma_start(out=sin_sb[:, 1024:2048], in_=sin_d[:, 1024:2048])
            nc.sync.dma_start(out=wv_sb[:, 0:4, :], in_=wvT_r[:, 0:4])
            nc.sync.dma_start(out=wv_sb[:, 4:8, :], in_=wvT_r[:, 4:8])
            nc.gpsimd.dma_start(out=x8_sb[:, 1, 0:2], in_=xP8_d[:, 1, 0:2])
            nc.gpsimd.dma_start(out=x8_sb[:, 1, 2:4], in_=xP8_d[:, 1, 2:4])
            x_second = load_x(1)
            nc.sync.dma_start(out=tri_sb[:], in_=tri_d[:])
            nc.gpsimd.dma_start(out=x8_sb[:, 2], in_=xP8_d[:, 2])
            x_third = load_x(2)
            nc.sync.dma_start(out=wo_sb[:, 0, :], in_=woT_r[:, 0])
            nc.sync.dma_start(out=wo_sb[:, 1, :], in_=woT_r[:, 1])
            nc.gpsimd.dma_start(out=x8_sb[:, 3], in_=xP8_d[:, 3])
            x_fourth = load_x(3)
            for i in range(NKT):
                nc.gpsimd.memset(vt[i][:, :, HDIM:HDIM + 1], 1.0)

            # ---- Phase A: fp8 DoubleRow q/k projections + RoPE; bf16 v ----
            def phase_a(sb, x_sb):
                ssl = slice(sb * 512, (sb + 1) * 512)
                for dtl in range(2):
                    dsl = slice(dtl * 128, (dtl + 1) * 128)
                    for w8, dest in ((wq8_sb, qT), (wk8_sb, kT)):
                        pp = pv_ps.tile([128, 512], f32, tag="ppv", name=f"pp{sb}{dtl}")
                        for g in range(4):
                            nc.tensor.matmul(
                                pp[:], w8[g // 2][:, g % 2, :, dsl], x8_sb[:, sb, g],
                                start=(g == 0), stop=(g == 3), perf_mode=DR,
                            )
                        raw = rope.tile([128, 512], dt, tag="raw")
                        nc.scalar.copy(raw[:], pp[:])
                        rot = rope.tile([128, 512], dt, tag="rot")
                        nc.vector.stream_shuffle(rot[:], raw[:], SHUF_MASK)
                        t1 = rope.tile([128, 512], dt, tag="t1")
                        nc.vector.tensor_mul(t1[:], raw[:], cos_sb[:, ssl])
                        t2 = rope.tile([128, 512], dt, tag="t2")
                        nc.vector.tensor_mul(t2[:], rot[:], sin_sb[:, ssl])
                        nc.vector.tensor_add(dest[dtl][:, ssl], t1[:], t2[:])
                # v projection (natural layout [s, d_local]); ones col pre-set
                def xe(et):
                    return x_sb[:, et, :]
                for st in range(4):
                    kt = sb * 4 + st
                    vp = pv_ps.tile([128, DL], f32, tag="ppv", name=f"vp{kt}")
                    for et in range(NET):
                        nc.tensor.matmul(
                            vp[:], xe(et)[:, st * 128:(st + 1) * 128],
                            wv_sb[:, et, :],
                            start=(et == 0), stop=(et == NET - 1),
                        )
                    nc.vector.tensor_copy(
                        vt[kt][:, :, 0:HDIM],
                        vp.rearrange("p (h d) -> p h d", h=GROUP_HEADS),
                    )

            # ---- Phase B: head pairs interleaved, PV lags one k-group ----
            def phase_b(qb):
                qsl = slice(qb * 512, (qb + 1) * 512)
                nkt = 4 * (qb + 1)
                for hp in range(2):
                    t = hp
                    pv = [
                        pv_ps.tile([128, 512], f32, tag="ppv", name=f"pv{qb}{hp}{h2}")
                        for h2 in range(2)
                    ]

                    def make_pv_stage(kp, pr, pv=pv, hp=hp):
                        def emit():
                            for h2 in range(2):
                                h = 2 * hp + h2
                                for j in range(2):
                                    kt = 2 * kp + j
                                    o = max(kt * 128 - qb * 512, 0)
                                    if kt >= 4 * qb:
                                        nc.vector.tensor_mul(
                                            pr[h2][:, j * 512 + o:j * 512 + o + 128],
                                            pr[h2][:, j * 512 + o:j * 512 + o + 128],
                                            tri_sb[:],
                                        )
                                    nc.tensor.matmul(
                                        pv[h2][0:65, o:512], vt[kt][:, h, :],
                                        pr[h2][:, j * 512 + o:(j + 1) * 512],
                                        start=(kt == 0), stop=(kt == nkt - 1),
                                    )
                        return emit

                    pv_prev = None
                    for kp in range(nkt // 2):
                        o0 = max(2 * kp * 128 - qb * 512, 0)
                        sc = []
                        for h2 in range(2):
                            psl = slice(h2 * 64, h2 * 64 + 64)
                            s_t = sc_ps.tile([128, 1024], f32, tag="sc")
                            for j in range(2):
                                kt = 2 * kp + j
                                o = max(kt * 128 - qb * 512, 0)
                                nc.tensor.matmul(
                                    s_t[:, j * 512 + o:(j + 1) * 512],
                                    kT[t][psl, kt * 128:(kt + 1) * 128],
                                    qT[t][psl, qb * 512 + o:(qb + 1) * 512],
                                    start=True, stop=True,
                                )
                            sc.append(s_t)
                        pr = []
                        for h2 in range(2):
                            p_t = probs.tile([128, 1024], dt, tag="pr")
                            nc.scalar.activation(
                                p_t[:, o0:], sc[h2][:, o0:], Exp, scale=ATTN_SCALE
                            )
                            pr.append(p_t)
                        if pv_prev is not None:
                            pv_prev()
                        pv_prev = make_pv_stage(kp, pr)
                    pv_prev()

                    for h2 in range(2):
                        sums = small.tile([1, 512], f32, tag="sums")
                        nc.vector.tensor_copy(sums[:], pv[h2][64:65, :])
                        inv = small.tile([1, 512], f32, tag="inv")
                        nc.vector.reciprocal_approx_fast(out=inv[:], in_=sums[:])
                        invb = small.tile([64, 512], f32, tag="invb")
                        nc.gpsimd.partition_broadcast(invb[:], inv[:])
                        nc.vector.tensor_mul(
                            outT[t][h2 * 64 + 0:h2 * 64 + 64, qsl],
                            pv[h2][0:64, :], invb[:]
                        )

            # ---- Phase C: output projection, batched [128, 1024] stores ----
            def phase_c(qt):
                qsl = slice(qt * 128, (qt + 1) * 128)
                fo = fopool.tile([128, E], dt, tag="fo")
                for eb in range(2):
                    esl = slice(eb * 512, (eb + 1) * 512)
                    f = pv_ps.tile([128, 512], f32, tag="ppv", name=f"f{qt}{eb}")
                    for dtl in range(2):
                        nc.tensor.matmul(
                            f[:], outT[dtl][:, qsl], wo_sb[:, dtl, esl],
                            start=(dtl == 0), stop=(dtl == 1),
                        )
                    if eb == 0:
                        nc.scalar.copy(fo[:, esl], f[:])
                    else:
                        nc.vector.tensor_copy(fo[:, esl], f[:])
                eng = nc.sync if qt % 2 == 0 else nc.gpsimd
                eng.dma_start(out=part_d[qsl, :], in_=fo[:])

            # ---- interleaved emission: A(sb) feeds B(qb=sb); C trails B ----
            phase_a(0, x_first)
            x_pre = [None, x_second, x_third, x_fourth]
            for blk in range(NQB):
                if blk + 1 < NSB:
                    phase_a(blk + 1, x_pre[blk + 1])
                phase_b(blk)
                if blk >= 1:
                    for qt in range(4 * (blk - 1), 4 * blk):
                        phase_c(qt)
            for qt in range(4 * (NQB - 1), NQT):
                phase_c(qt)

    nc.compile()
    return nc


def _host_tables():
    inv_freq = 1.0 / (ROPE_BASE ** (np.arange(HALF, dtype=np.float32) / HALF))
    angles = np.arange(S, dtype=np.float32)[:, None] * inv_freq[None, :]  # [S, 32]
    c = np.cos(angles).T.astype(np.float32)  # [32, S]
    s = np.sin(angles).T.astype(np.float32)
    cos64 = np.concatenate([c[0:16], c[0:16], c[16:32], c[16:32]], axis=0)
    sin64 = np.concatenate([-s[0:16], s[0:16], -s[16:32], s[16:32]], axis=0)
    cosP = np.tile(cos64, (2, 1))  # [128, S]
    sinP = np.tile(sin64, (2, 1))
    tri = (np.arange(128)[None, :] >= np.arange(128)[:, None]).astype(np.float32)
    return cosP, sinP, np.ascontiguousarray(tri)


def kernel(x, Wq, Wk, Wv, Wo):
    import ml_dtypes
    from concourse.bass_utils import run_bass_kernel_spmd

    x = np.asarray(x, dtype=np.float32)
    Wq = np.asarray(Wq, dtype=np.float32)
    Wk = np.asarray(Wk, dtype=np.float32)
    Wv = np.asarray(Wv, dtype=np.float32)
    Wo = np.asarray(Wo, dtype=np.float32)

    if "nc" not in _CACHE:
        _CACHE["nc"] = _build_nc()
    nc = _CACHE["nc"]

    np_bf = ml_dtypes.bfloat16
    np_f8 = ml_dtypes.float8_e4m3fn

    def cvt(a):
        return np.ascontiguousarray(a.astype(np_bf))

    # RoPE 16-interleave permutation within each head:
    # [evens 0:32:2 of pairs 0-15, odds, evens of pairs 16-31, odds]
    e = np.arange(0, HDIM, 2)
    o = np.arange(1, HDIM, 2)
    perm = np.concatenate([e[0:16], o[0:16], e[16:32], o[16:32]])
    full_perm = np.concatenate([h * HDIM + perm for h in range(N_HEADS)])
    Wq_p = Wq[full_perm]
    Wk_p = Wk[full_perm]

    cosP, sinP, tri = _host_tables()

    def packx_bf(xt):
        # [E, S] -> [128, 4, 2, 4, 512]: (p, sb, half, et4, s)
        return np.ascontiguousarray(
            xt.reshape(2, 4, 128, 4, 512).transpose(2, 3, 0, 1, 4))

    def packx_f8(xt):
        # [E, S] -> [128, 4, 4, 2, 512]: (p, sb, g, i, s); e = g*256 + i*128 + p
        return np.ascontiguousarray(
            xt.reshape(4, 2, 128, 4, 512).transpose(2, 3, 0, 1, 4).astype(np_f8))

    def packw8(w_local):
        # w_local [DL, E] -> wT [E, DL] -> [128, 4, 2, DL]
        return np.ascontiguousarray(
            w_local.T.reshape(4, 2, 128, DL).transpose(2, 0, 1, 3).astype(np_f8))

    xT = [np.ascontiguousarray(x[b].T) for b in range(B)]
    xbf = [packx_bf(cvt(xT[b])) for b in range(B)]
    xf8 = [packx_f8(xT[b]) for b in range(B)]
    cosP, sinP, tri = cvt(cosP), cvt(sinP), cvt(tri)

    in_maps = []
    for c in range(N_CORES):
        b, g = c // 4, c % 4
        dsl = slice(g * DL, (g + 1) * DL)
        in_maps.append({
            "xP8": xf8[b],
            "xP": xbf[b],
            "wq8": packw8(Wq_p[dsl]),
            "wk8": packw8(Wk_p[dsl]),
            "wvT": cvt(Wv[dsl].T),
            "woT": cvt(Wo[:, dsl].T),
            "cosP": cosP,
            "sinP": sinP,
            "tri": tri,
        })

    trace = bool(int(os.environ.get("ANT_KERNEL_TRACE", "0")))
    res = None
    for attempt in range(3):
        try:
            res = run_bass_kernel_spmd(
                nc, in_maps, core_ids=list(range(N_CORES)), trace=trace
            )
            break
        except Exception:
            if attempt == 2:
                raise
            import time as _time
            _time.sleep(20)
    _CACHE["last_exec_time_ns"] = res.exec_time_ns
    _CACHE["last_res"] = res

    out = np.zeros((B, S, E), dtype=np.float32)
    for c in range(N_CORES):
        out[c // 4] += np.asarray(res.results[c]["part"], dtype=np.float32)
    return out
